# revision 12
# baseline (speedup 1.0000x reference)
"""BotRGCN (4 shared RGCN layers) on 8 TRN2 NeuronCores via Bass/Tile.

Strategy (sharding_hint): nodes sharded across 8 cores (6250 each, padded to
6656 = 13*512); edges partitioned by destination core and sorted by
(dst_local, rel) segment; per layer the row-major fp16 x table is
replicated to every core's DRAM via 2 half-table AllGathers that are fired
mid-chunk-loop of the PRODUCING layer (so they hide under gather/compute of
the previous layer); each core then dma_gathers its edges' source rows
(4 SWDGE queues, round-robin) and computes segment means via PE matmuls
against 0/1*(1/cnt) membership matrices built host-side (static graph; the
tiling is baked into the compiled program, identical across cores).
Stream-0 (table half 0) gathers are issued 3 chunks ahead of stream-1 so
the half-1 AllGather (fired at the producer's last chunk) stays hidden.

Self-contained: hardcodes all shapes from the problem spec.
"""
import os
import time

import numpy as np

import concourse.bacc as bacc
import concourse.bass as bass
import concourse.mybir as mybir
import concourse.tile as tile
from concourse.bass_utils import run_bass_kernel_spmd
from concourse.masks import make_identity

# ---------------- problem constants (hardcoded from spec) ----------------
NCORES = 8
N = 50000
E = 800000
R = 5
D = 128
FIN = 768 + 768 + 6 + 11          # 1553 concat input features
FINP = 13 * 128                   # padded to 1664
NLOC = N // NCORES                # 6250
CHUNK = 512                       # nodes per chunk
NCHUNK = 13
NPAD = NCHUNK * CHUNK             # 6656 padded nodes/core
BANK = 512                        # segment columns per PSUM bank
BANKS_PER_CHUNK = CHUNK * R // BANK   # 5
NBANK = NCHUNK * BANKS_PER_CHUNK  # 65
NSEG = NPAD * R                   # 33280 dense segment grid per core
HALFROW = NPAD // 2               # 3328: rows per half-table shard
NTABH = NCORES * HALFROW          # 26624 rows per half table (< 32768)
SLOTS = 128                       # edge slots per tile
SUBT = 8                          # gather tiles per SWDGE call (ring cap)
LOOKAHEAD = 3                     # chunks of stream-0 gather lookahead
NLAYER = int(os.environ.get("KB_LAYERS", "4"))
# producing-chunk -> half-table AllGather fire points (h0 ready after c=6,
# fired with slack at 8; h1 ready only at the last chunk)
FIRES = {8: 0, NCHUNK - 1: 1}

F16 = mybir.dt.float16
F32 = mybir.dt.float32
I16 = mybir.dt.int16

_CACHE = {}


# ---------------- host-side graph preprocessing ----------------
def _plan_graph(edge_index, edge_type):
    """Build per-core tile structure. Span layout is shared by all cores
    (SPMD: one program), per-core data (idx, M) differs."""
    src = np.asarray(edge_index[0], dtype=np.int64)
    dst = np.asarray(edge_index[1], dtype=np.int64)
    et = np.asarray(edge_type, dtype=np.int64)

    core = dst // NLOC
    col = (dst % NLOC) * R + et                       # 0..31249
    src_core = src // NLOC
    src_loc = src % NLOC
    stream = (src_loc >= HALFROW).astype(np.int64)    # src half
    # row index within the half table
    src_pad = src_core * HALFROW + (src_loc - stream * HALFROW)

    # per (core, stream): edges sorted by col
    edges = {}
    counts = np.zeros((NCORES, 2, NSEG), dtype=np.int64)
    for k in range(NCORES):
        for s in range(2):
            m = (core == k) & (stream == s)
            c = col[m]
            o = np.argsort(c, kind="stable")
            edges[(k, s)] = (c[o], src_pad[m][o])
            np.add.at(counts[k, s], c[o], 1)

    cnt_total = counts.sum(axis=1)                    # [NCORES, NSEG]
    invc = 1.0 / np.maximum(cnt_total, 1.0)          # per core

    # static spans per (stream, bank): greedy, max-over-cores count <= SLOTS
    spans = {0: [], 1: []}                            # spans[s][b] = [widths]
    for s in range(2):
        for b in range(NBANK):
            base = b * BANK
            cc = counts[:, s, base:base + BANK]       # [NCORES, BANK]
            assert cc.max(initial=0) <= SLOTS, "single segment exceeds tile"
            widths = []
            run = np.zeros(NCORES, dtype=np.int64)
            w = 0
            for j in range(BANK):
                if (run + cc[:, j]).max() > SLOTS:
                    widths.append(w)
                    run[:] = 0
                    w = 0
                run += cc[:, j]
                w += 1
            widths.append(w)
            spans[s].append(widths)

    ntiles = {s: [len(spans[s][b]) for b in range(NBANK)] for s in range(2)}
    # gather-call grouping: one call per (stream, chunk)
    call_tiles = {s: [sum(ntiles[s][c * BANKS_PER_CHUNK + b]
                          for b in range(BANKS_PER_CHUNK))
                      for c in range(NCHUNK)] for s in range(2)}
    tot_tiles = {s: sum(ntiles[s]) for s in range(2)}

    # per-core data: gather idx (wrapped int16) + M matrices
    gidx = {s: np.zeros((NCORES, 128, tot_tiles[s] * SLOTS // 16), np.int16)
            for s in range(2)}
    mmat = np.zeros((NCORES, 128, 2 * NBANK * BANK), np.float16)
    for k in range(NCORES):
        for s in range(2):
            cols_e, srcp_e = edges[(k, s)]
            idx_vals = srcp_e
            flat_idx = np.zeros(tot_tiles[s] * SLOTS, np.int16)
            tglob = 0
            for b in range(NBANK):
                base = b * BANK
                lo = 0
                for w in spans[s][b]:
                    e0 = np.searchsorted(cols_e, base + lo)
                    e1 = np.searchsorted(cols_e, base + lo + w)
                    nslot = e1 - e0
                    assert nslot <= SLOTS
                    flat_idx[tglob * SLOTS:tglob * SLOTS + nslot] = \
                        idx_vals[e0:e1]
                    mcol = (s * NBANK + b) * BANK + (cols_e[e0:e1] - base)
                    mmat[k, np.arange(nslot), mcol] = \
                        invc[k][cols_e[e0:e1]].astype(np.float16)
                    lo += w
                    tglob += 1
            # wrap: element i -> [i%16, i//16], replicated across 8 groups
            wr = flat_idx.reshape(-1, 16).T            # [16, ntot*8]
            gidx[s][k] = np.tile(wr, (8, 1))
    return dict(spans=spans, ntiles=ntiles, call_tiles=call_tiles,
                tot_tiles=tot_tiles, gidx=gidx, mmat=mmat)


# ---------------- device program ----------------
def _build_nc(plan):
    nc = bacc.Bacc("TRN2", target_bir_lowering=False, debug=False,
                   num_devices=NCORES, num_swdge_queues=4,
                   dynamic_dma_scratch_size=32768)
    spans, ntiles = plan["spans"], plan["ntiles"]
    call_tiles, tot_tiles = plan["call_tiles"], plan["tot_tiles"]

    # inputs (per core)
    featT = nc.dram_tensor("featT", [FINP, NPAD], F16, kind="ExternalInput")
    w_all = nc.dram_tensor("w_all", [128, 13 * 128], F16, kind="ExternalInput")
    b_x0 = nc.dram_tensor("b_x0", [128, 1], F32, kind="ExternalInput")
    w_in = nc.dram_tensor("w_in", [128, 128], F16, kind="ExternalInput")
    b_in = nc.dram_tensor("b_in", [128, 1], F32, kind="ExternalInput")
    relw = nc.dram_tensor("relw", [128, R * 128], F16, kind="ExternalInput")
    rootw = nc.dram_tensor("rootw", [128, 128], F16, kind="ExternalInput")
    rgcn_b = nc.dram_tensor("rgcn_b", [128, 1], F32, kind="ExternalInput")
    wo1 = nc.dram_tensor("wo1", [128, 128], F16, kind="ExternalInput")
    b_o1 = nc.dram_tensor("b_o1", [128, 1], F32, kind="ExternalInput")
    wo2 = nc.dram_tensor("wo2", [128, 2], F16, kind="ExternalInput")
    b_o2 = nc.dram_tensor("b_o2", [2, 1], F32, kind="ExternalInput")
    gidxA = nc.dram_tensor("gidxA", [128, tot_tiles[0] * 8], I16,
                           kind="ExternalInput")
    gidxB = nc.dram_tensor("gidxB", [128, tot_tiles[1] * 8], I16,
                           kind="ExternalInput")
    mmat = nc.dram_tensor("mmat", [128, 2 * NBANK * BANK], F16,
                          kind="ExternalInput")
    outT = nc.dram_tensor("outT", [2, NPAD], F32, kind="ExternalOutput")

    with tile.TileContext(nc) as tc:
        with (
            tc.tile_pool(name="const", bufs=1) as constp,
            tc.tile_pool(name="xt", bufs=2) as xtp,
            tc.tile_pool(name="feat", bufs=3) as featp,
            tc.tile_pool(name="gb0", bufs=(LOOKAHEAD + 1) * 7) as gbp0,
            tc.tile_pool(name="gb1", bufs=12) as gbp1,
            tc.tile_pool(name="msb", bufs=3) as msbp,
            tc.tile_pool(name="stile", bufs=2) as stp,
            tc.tile_pool(name="small", bufs=3) as smallp,
            tc.tile_pool(name="pbank", bufs=3, space="PSUM") as pbank,
            tc.tile_pool(name="pbig", bufs=2, space="PSUM") as pbig,
            tc.tile_pool(name="ptp", bufs=2, space="PSUM") as ptpp,
            tc.tile_pool(name="dram", bufs=1, space="DRAM") as dramp,
            tc.tile_pool(name="shared", bufs=1, space="DRAM") as sharedp,
        ):
            gbp = {0: gbp0, 1: gbp1}

            # ---- resident constants ----
            def load_const(t, shape, dt):
                s = constp.tile(shape, dt, tag=t.name)
                nc.sync.dma_start(s[:], t[:])
                return s
            w_all_s = load_const(w_all, [128, 13 * 128], F16)
            b_x0_s = load_const(b_x0, [128, 1], F32)
            w_in_s = load_const(w_in, [128, 128], F16)
            b_in_s = load_const(b_in, [128, 1], F32)
            relw_s = load_const(relw, [128, R * 128], F16)
            rootw_s = load_const(rootw, [128, 128], F16)
            rgcn_b_s = load_const(rgcn_b, [128, 1], F32)
            wo1_s = load_const(wo1, [128, 128], F16)
            b_o1_s = load_const(b_o1, [128, 1], F32)
            wo2_s = load_const(wo2, [128, 2], F16)
            b_o2_s = load_const(b_o2, [2, 1], F32)
            gidx_s = [load_const(gidxA, [128, tot_tiles[0] * 8], I16),
                      load_const(gidxB, [128, tot_tiles[1] * 8], I16)]
            ident = constp.tile([128, 128], F16, tag="ident")
            make_identity(nc, ident[:])

            gq = [0]                  # round-robin SWDGE queue counter
            tables = {}               # layer -> [tb_half0, tb_half1]

            def start_tables(layer):
                tb = [sharedp.tile([NTABH, D], F16, addr_space="Shared",
                                   name=f"table{layer}_{s}",
                                   tag=f"table{layer}_{s}") for s in range(2)]
                tst = xtp.tile([128, NPAD], F16, tag="tstage")
                tables[layer] = tb
                return tb, tst

            def produce_chunk(layer, src, c, tb, tst):
                """Transpose chunk c of src into the row-major staging table;
                fire quarter-shard DMA + AllGather at the FIRES points."""
                for j in range(4 * c, 4 * c + 4):
                    pt = ptpp.tile([128, 128], F16, space="PSUM", tag="ptp")
                    nc.tensor.transpose(pt[:], src[:, j * 128:(j + 1) * 128],
                                        ident[:])
                    nc.vector.tensor_copy(tst[:, j * 128:(j + 1) * 128],
                                          pt[:])
                h = FIRES.get(c)
                if h is not None:
                    tq = dramp.tile([HALFROW, D], F16, tag=f"tsh{layer}_{h}",
                                    name=f"tsh{layer}_{h}")
                    nc.sync.dma_start(
                        tq[:].rearrange("(j p) d -> p j d", p=128),
                        tst[:, h * HALFROW:(h + 1) * HALFROW].rearrange(
                            "p (j d) -> p j d", d=D))
                    nc.gpsimd.collective_compute(
                        "AllGather", mybir.AluOpType.bypass,
                        replica_groups=[list(range(NCORES))],
                        ins=[tq[:].opt()], outs=[tb[h][:].opt()])

            def issue_gathers(layer, s, c, goff):
                """SWDGE gather sub-calls for (stream, chunk); <=SUBT tiles."""
                tc_s = call_tiles[s][c]
                view = tables[layer][s][:]
                subs = []
                for t0 in range(0, tc_s, SUBT):
                    nt = min(SUBT, tc_s - t0)
                    gb = gbp[s].tile([128, SUBT, D], F16, tag=f"gb{s}")
                    ni = nt * SLOTS
                    nc.gpsimd.dma_gather(
                        gb[:, :nt, :], view,
                        gidx_s[s][:, goff[s]:goff[s] + ni // 16],
                        ni, ni, D, queue_num=gq[0] % 4, single_packet=True)
                    gq[0] += 1
                    goff[s] += ni // 16
                    subs.append(gb)
                return subs

            # ---- input projection -> xT [128, NPAD] fp16 (+ layer-0 table)
            xT = xtp.tile([128, NPAD], F16, tag="xT")
            tb0, tst0 = start_tables(0)
            for c in range(NCHUNK):
                cs = slice(c * CHUNK, (c + 1) * CHUNK)
                p0 = pbig.tile([128, CHUNK], F32, space="PSUM", tag="pbig")
                for f in range(13):
                    ft = featp.tile([128, CHUNK], F16, tag="feat")
                    nc.sync.dma_start(ft[:], featT[f * 128:(f + 1) * 128, cs])
                    nc.tensor.matmul(p0[:],
                                     lhsT=w_all_s[:, f * 128:(f + 1) * 128],
                                     rhs=ft[:], start=(f == 0), stop=(f == 12))
                x0 = smallp.tile([128, CHUNK], F16, tag="x0")
                nc.scalar.activation(x0[:], p0[:],
                                     mybir.ActivationFunctionType.Lrelu,
                                     bias=b_x0_s[:], scale=1.0, alpha=0.01)
                p1 = pbig.tile([128, CHUNK], F32, space="PSUM", tag="pbig")
                nc.tensor.matmul(p1[:], lhsT=w_in_s[:], rhs=x0[:],
                                 start=True, stop=True)
                nc.scalar.activation(xT[:, cs], p1[:],
                                     mybir.ActivationFunctionType.Lrelu,
                                     bias=b_in_s[:], scale=1.0, alpha=0.01)
                produce_chunk(0, xT, c, tb0, tst0)

            # ---- RGCN layers ----
            for layer in range(NLAYER):
                if layer + 1 < NLAYER:
                    tbn, tstn = start_tables(layer + 1)
                xTn = xtp.tile([128, NPAD], F16, tag="xT")
                goff = {0: 0, 1: 0}   # gather idx cursor per stream
                pending = {}
                for c in range(min(LOOKAHEAD, NCHUNK)):
                    pending[(0, c)] = issue_gathers(layer, 0, c, goff)
                for c in range(NCHUNK):
                    if c + LOOKAHEAD < NCHUNK:
                        pending[(0, c + LOOKAHEAD)] = issue_gathers(
                            layer, 0, c + LOOKAHEAD, goff)
                    pending[(1, c)] = issue_gathers(layer, 1, c, goff)
                    gtiles = {s: pending.pop((s, c)) for s in range(2)}
                    st = stp.tile([128, CHUNK * R], F16, tag="stile")
                    for b in range(BANKS_PER_CHUNK):
                        bg = c * BANKS_PER_CHUNK + b
                        pb = pbank.tile([128, BANK], F32, space="PSUM",
                                        tag="pbank")
                        n_mm = len(spans[0][bg]) + len(spans[1][bg])
                        i_mm = 0
                        for s in range(2):
                            ms = msbp.tile([128, BANK], F16, tag="msb")
                            nc.sync.dma_start(
                                ms[:], mmat[:, (s * NBANK + bg) * BANK:
                                            (s * NBANK + bg + 1) * BANK])
                            lo = 0
                            # local tile index within this chunk's call
                            tloc = sum(ntiles[s][c * BANKS_PER_CHUNK + bb]
                                       for bb in range(b))
                            for w in spans[s][bg]:
                                nc.tensor.matmul(
                                    pb[:, lo:lo + w],
                                    lhsT=gtiles[s][tloc // 8][:, tloc % 8, :],
                                    rhs=ms[:, lo:lo + w],
                                    start=(i_mm == 0),
                                    stop=(i_mm == n_mm - 1))
                                lo += w
                                tloc += 1
                                i_mm += 1
                            assert lo == BANK
                        assert i_mm == n_mm
                        nc.vector.tensor_copy(st[:, b * BANK:(b + 1) * BANK],
                                              pb[:])
                    # phase 2: per-relation + root matmuls
                    cs = slice(c * CHUNK, (c + 1) * CHUNK)
                    po = pbig.tile([128, CHUNK], F32, space="PSUM", tag="pbig")
                    str_ap = st[:].rearrange("p (n r) -> p r n", r=R)
                    for r in range(R):
                        nc.tensor.matmul(po[:],
                                         lhsT=relw_s[:, r * 128:(r + 1) * 128],
                                         rhs=str_ap[:, r, :],
                                         start=(r == 0), stop=False)
                    nc.tensor.matmul(po[:], lhsT=rootw_s[:], rhs=xT[:, cs],
                                     start=False, stop=True)
                    nc.scalar.activation(xTn[:, cs], po[:],
                                         mybir.ActivationFunctionType.Identity,
                                         bias=rgcn_b_s[:], scale=1.0)
                    if layer + 1 < NLAYER:
                        produce_chunk(layer + 1, xTn, c, tbn, tstn)
                xT = xTn

            # ---- output head ----
            for c in range(NCHUNK):
                cs = slice(c * CHUNK, (c + 1) * CHUNK)
                p1 = pbig.tile([128, CHUNK], F32, space="PSUM", tag="pbig")
                nc.tensor.matmul(p1[:], lhsT=wo1_s[:], rhs=xT[:, cs],
                                 start=True, stop=True)
                h = smallp.tile([128, CHUNK], F16, tag="x0")
                nc.scalar.activation(h[:], p1[:],
                                     mybir.ActivationFunctionType.Lrelu,
                                     bias=b_o1_s[:], scale=1.0, alpha=0.01)
                p2 = ptpp.tile([2, CHUNK], F32, space="PSUM", tag="ptp")
                nc.tensor.matmul(p2[:], lhsT=wo2_s[:], rhs=h[:],
                                 start=True, stop=True)
                ot = smallp.tile([2, CHUNK], F32, tag="ot")
                nc.scalar.activation(ot[:], p2[:],
                                     mybir.ActivationFunctionType.Identity,
                                     bias=b_o2_s[:], scale=1.0)
                nc.sync.dma_start(outT[:, cs], ot[:])

    nc.compile()
    return nc


# ---------------- host wrapper ----------------
def _pack_inputs(inputs, plan):
    f16 = np.float16
    des, tweet = inputs["des"], inputs["tweet"]
    num_prop, cat_prop = inputs["num_prop"], inputs["cat_prop"]

    w_blk = np.zeros((FINP, 128), np.float32)
    w_blk[0:768, 0:32] = inputs["W_des"]
    w_blk[768:1536, 32:64] = inputs["W_tw"]
    w_blk[1536:1542, 64:96] = inputs["W_np"]
    w_blk[1542:1553, 96:128] = inputs["W_cp"]
    # pack lhsT blocks: [128, 13*128], block f = rows [f*128,(f+1)*128)
    w_all = np.concatenate([w_blk[f * 128:(f + 1) * 128, :]
                            for f in range(13)], axis=1).astype(f16)
    b_x0 = np.concatenate([inputs["b_des"], inputs["b_tw"],
                           inputs["b_np"], inputs["b_cp"]]
                          ).astype(np.float32).reshape(128, 1)
    relw = np.concatenate([inputs["rel_w"][r] for r in range(R)],
                          axis=1).astype(f16)

    in_maps = []
    for k in range(NCORES):
        rows = slice(k * NLOC, (k + 1) * NLOC)
        feat = np.zeros((FINP, NPAD), f16)
        feat[0:768, :NLOC] = des[rows].T
        feat[768:1536, :NLOC] = tweet[rows].T
        feat[1536:1542, :NLOC] = num_prop[rows].T
        feat[1542:1553, :NLOC] = cat_prop[rows].T
        m = {
            "featT": feat,
            "w_all": w_all,
            "b_x0": b_x0,
            "w_in": inputs["W_in"].astype(f16),
            "b_in": inputs["b_in"].astype(np.float32).reshape(128, 1),
            "relw": relw,
            "rootw": inputs["root_w"].astype(f16),
            "rgcn_b": inputs["rgcn_b"].astype(np.float32).reshape(128, 1),
            "wo1": inputs["W_o1"].astype(f16),
            "b_o1": inputs["b_o1"].astype(np.float32).reshape(128, 1),
            "wo2": inputs["W_o2"].astype(f16),
            "b_o2": inputs["b_o2"].astype(np.float32).reshape(2, 1),
            "gidxA": plan["gidx"][0][k],
            "gidxB": plan["gidx"][1][k],
            "mmat": plan["mmat"][k],
        }
        in_maps.append(m)
    return in_maps


def _get_compiled(edge_index, edge_type):
    key = hash((np.asarray(edge_index).tobytes(),
                np.asarray(edge_type).tobytes()))
    if key not in _CACHE:
        t0 = time.time()
        plan = _plan_graph(edge_index, edge_type)
        t1 = time.time()
        nc = _build_nc(plan)
        t2 = time.time()
        print(f"[kernel] plan {t1-t0:.0f}s, build+compile {t2-t1:.0f}s",
              flush=True)
        _CACHE[key] = (nc, plan)
    return _CACHE[key]


def kernel(trace=False, **inputs):
    nc, plan = _get_compiled(inputs["edge_index"], inputs["edge_type"])
    in_maps = _pack_inputs(inputs, plan)
    t0 = time.time()
    res = run_bass_kernel_spmd(nc, in_maps, list(range(NCORES)), trace=trace)
    print(f"[kernel] run {time.time()-t0:.0f}s", flush=True)
    out = np.zeros((N, 2), np.float32)
    for k in range(NCORES):
        out[k * NLOC:(k + 1) * NLOC] = res.results[k]["outT"][:, :NLOC].T
    if trace:
        return out, res
    return out
